# revision 16
# baseline (speedup 1.0000x reference)
"""DecomposedSTFT power-spectrum kernel for 8 Trainium2 NeuronCores.

Reference computation:
    padded = reflect_pad(audio, 512)                       # [15361024]
    frame_t = padded[512*t : 512*t + 1024], t = 0..30000   # hop 512, win 1024
    ft[ch, t] = sum_n basis[ch, n] * frame_t[n]            # basis [1026, 1024]
    out[k, t] = ft[k, t]^2 + ft[513+k, t]^2                # k = 0..512

Structure exploited:
  * Window fold: basis[ch] = fb[ch] * win with periodic Hann win, so
    win[n+512] = 1 - win[n] and fb[ch, n+512] = (-1)^k fb[ch, n].  Hence
        ft[ch] = sum_{n<512} fb[ch, n] * (u[n] if k even else d[n])
        u_t[n] = win[n] x_t[n] + (1-win[n]) x_t[n+512]
        d_t[n] = win[n] x_t[n] - (1-win[n]) x_t[n+512]
    halving the matmul contraction from 1024 to 512.  fb is recovered from
    the input basis exactly: fb[ch,n] = basis[ch,n] +- basis[ch,n+512].
    u/d are built on-device (ACT scale-copy + DVE fused multiply-adds) so
    the audio crosses PCIe/HBM exactly once.
  * imag rows 0 and 512 are identically zero, so the 1026 channels are
    exactly 1024 nonzero rows = 8 M-tiles of 128, grouped by bin parity:
    mt 0-3 read u (even bins), mt 4-7 read d (odd bins).  Power =
    square + square partitionwise with two single-row fixups (k=0, k=512).
  * hop = win/2: audio laid out as 512-sample blocks (transposed host-side
    to [512, n_blocks]) makes x[n+512] a 1-column shift of x[n].
  * float32r matmuls: 1 column/cycle for even N >= 256, full fp32 numerics.
  * DMAs carry a ~2us serialized fixed cost each, so they are batched:
    audio arrives in 4 whole-iteration transfers, outputs leave in
    2-chunk-wide strips, split across both HWDGE rings (sync + scalar).

Sharding: frames split across 8 cores (3760 frames/core, surplus frames
cropped on gather); forward_basis replicated.
"""

import contextlib

import numpy as np

import concourse.bass as bass
import concourse.mybir as mybir
import concourse.tile as tile
from concourse import bacc
from concourse.bass_utils import run_bass_kernel_spmd

FL = 1024
HOP = 512
CUT = 513  # output rows
PAD = 512
L = 15360000
T_FRAMES = 30001

N_CORES = 8
F = 3760            # frames per core (8*3760 = 30080 >= 30001)
CHUNK = 470         # frames per N-chunk; must be EVEN (f32r ISA restriction),
                    # >= 256 (f32r full rate), <= 512 (one PSUM bank)
N_CHUNKS = 8
GRP = 4             # chunks per output strip (out-DMA batching)
KC = 4              # K chunks of 128 (folded contraction dim 512)
MT = 8              # M tiles of 128 (1024 nonzero basis rows)

F32R = mybir.dt.float32r
F32 = mybir.dt.float32

# (mt_a, mt_b, out_row_offset): power pairs and the DRAM rows they produce
# (row = offset + 2*p).  mt 0/1 = real even bins, 2/3 = [Nyq, imag even],
# 4/5 = real odd, 6/7 = imag odd.
PAIRS = [(0, 2, 0), (1, 3, 256), (4, 6, 1), (5, 7, 257)]


def build_stft_nc(F=F, chunk=CHUNK, n_chunks=N_CHUNKS, repeat=1):
    """Bass program: audio_t [512, F+1] x w [128, 4096] -> out [513, F].

    repeat > 1 wraps the compute in a hardware loop redoing identical work;
    used only for wall-clock benchmarking (Dwall between two repeat counts
    isolates per-iteration HW time from dispatch/transfer overhead).
    """
    assert chunk * n_chunks == F
    nc = bacc.Bacc("TRN2", target_bir_lowering=False, debug=False)
    audio_t = nc.dram_tensor("audio_t", [512, F + 1], F32R, kind="ExternalInput")
    w_dram = nc.dram_tensor("w", [128, KC * MT * 128], F32R, kind="ExternalInput")
    winv_dram = nc.dram_tensor("winv", [128, 12], F32, kind="ExternalInput")
    out_dram = nc.dram_tensor("out", [CUT, F], F32, kind="ExternalOutput")

    def out_ap(row_off, row_step, nrows, col_off, ncols):
        return bass.AP(
            tensor=out_dram,
            offset=row_off * F + col_off,
            ap=[[row_step * F, nrows], [1, ncols]],
        )

    with tile.TileContext(nc) as tc:
        with (
            tc.tile_pool(name="wpool", bufs=1) as wpool,
            tc.tile_pool(name="apool", bufs=3) as apool,
            tc.tile_pool(name="zpool", bufs=3) as zpool,
            tc.tile_pool(name="udpool", bufs=3) as udpool,
            tc.tile_pool(name="spool", bufs=8) as spool,
            tc.tile_pool(name="opool", bufs=2) as opool,
            tc.tile_pool(name="npool", bufs=1) as npool,
            tc.tile_pool(name="psum", bufs=8, space="PSUM") as pp,
        ):
            winv = wpool.tile([128, 12], F32)
            nc.sync.dma_start(out=winv, in_=winv_dram[:, :])
            wt = wpool.tile([128, KC * MT * 128], F32R)
            nc.sync.dma_start(out=wt, in_=w_dram[:, :])

            loop_ctx = (
                tc.For_i(0, repeat, 1, hint_engines=(mybir.EngineType.PE,))
                if repeat > 1
                else contextlib.nullcontext()
            )
            with loop_ctx:
                nyq_full = npool.tile([1, F], F32, tag="nyq")
                ostrip = {}
                for n in range(n_chunks):
                    g = n % GRP  # position within the output strip
                    if g == 0:
                        for pi in range(4):
                            ostrip[pi] = opool.tile(
                                [128, GRP * chunk], F32,
                                tag=f"o{pi}", name=f"ostrip{pi}",
                            )

                    c0 = n * chunk
                    ud = {}
                    for kc in range(KC):
                        x = apool.tile([128, chunk + 1], F32R, tag=f"x{kc}",
                                       name=f"x{kc}")
                        nc.sync.dma_start(
                            out=x,
                            in_=audio_t[
                                kc * 128 : (kc + 1) * 128, c0 : c0 + chunk + 1
                            ],
                        )
                        x0 = x[:, 0:chunk]
                        x1 = x[:, 1 : chunk + 1]
                        # z = win * x[n]; u = z + (1-win) x[n+512];
                        # d = z - (1-win) x[n+512]
                        z = zpool.tile([128, chunk], F32R, tag="z")
                        nc.scalar.activation(
                            z,
                            x0,
                            mybir.ActivationFunctionType.Copy,
                            scale=winv[:, kc : kc + 1],
                        )
                        u = udpool.tile([128, chunk], F32R, tag=f"u{kc}")
                        d = udpool.tile([128, chunk], F32R, tag=f"d{kc}")
                        nc.vector.scalar_tensor_tensor(
                            u, x1, winv[:, 4 + kc : 5 + kc], z,
                            mybir.AluOpType.mult, mybir.AluOpType.add,
                        )
                        nc.vector.scalar_tensor_tensor(
                            d, x1, winv[:, 8 + kc : 9 + kc], z,
                            mybir.AluOpType.mult, mybir.AluOpType.add,
                        )
                        ud[kc] = (u, d)

                    sq = [None] * MT
                    for pi, (ma, mb, row_off) in enumerate(PAIRS):
                        for mt in (ma, mb):
                            p = pp.tile([128, chunk], F32)
                            for kc in range(KC):
                                rhs = ud[kc][0] if mt < 4 else ud[kc][1]
                                nc.tensor.matmul(
                                    p,
                                    wt[
                                        :,
                                        (kc * MT + mt) * 128 : (kc * MT + mt + 1) * 128,
                                    ],
                                    rhs,
                                    start=(kc == 0),
                                    stop=(kc == KC - 1),
                                )
                            s = spool.tile([128, chunk], F32, tag="sq")
                            nc.scalar.activation(
                                s, p, mybir.ActivationFunctionType.Square
                            )
                            sq[mt] = s

                        o = ostrip[pi][:, g * chunk : (g + 1) * chunk]
                        nc.vector.tensor_add(o, sq[ma], sq[mb])
                        if row_off == 0:
                            # row 0: imag_0 == 0; slot (2, p=0) actually holds
                            # the Nyquist real row -> out[0] = real_0^2 only
                            nc.vector.tensor_copy(o[0:1, :], sq[0][0:1, :])
                            # row 512: Nyquist power = (fb_512 . u)^2
                            nc.vector.tensor_copy(
                                nyq_full[:, c0 : c0 + chunk], sq[2][0:1, :]
                            )
                        if g == GRP - 1 or n == n_chunks - 1:
                            # strip complete: one DMA per pair on the SWDGE
                            # (gpsimd) queue so blocked output waits never
                            # stall input loads or compute engines
                            nc.gpsimd.dma_start(
                                out=out_ap(
                                    row_off, 2, 128, (n - g) * chunk,
                                    (g + 1) * chunk,
                                ),
                                in_=ostrip[pi][:, 0 : (g + 1) * chunk],
                            )
                nc.gpsimd.dma_start(out=out_ap(512, 1, 1, 0, F), in_=nyq_full)
    nc.compile()
    return nc


def _win512():
    n = np.arange(512)
    return (0.5 * (1.0 - np.cos(2.0 * np.pi * n / FL))).astype(np.float32)


def pack_weights(forward_basis):
    """[1026, 1, 1024] conv basis -> [128, 4096] folded lhsT tiles.

    fb[ch, n] = basis[ch, n] + (-1)^k basis[ch, n+512]  (exact unfold of the
    periodic-Hann window).  M layout groups bins by parity; imag rows 0/512
    (identically zero) are dropped and the Nyquist real row takes slot
    (mt=2, p=0).
    """
    basis = np.asarray(forward_basis, dtype=np.float32)[:, 0, :]  # [1026, 1024]
    k_of_ch = np.concatenate([np.arange(513), np.arange(513)])  # channel -> bin
    sign = np.where(k_of_ch % 2 == 0, 1.0, -1.0).astype(np.float32)
    fb = basis[:, :512] + sign[:, None] * basis[:, 512:]  # [1026, 512]

    ev = np.arange(0, 256, 2)
    od = np.arange(1, 256, 2)
    ch_tiles = [
        ev,                                     # mt0: real k = 0,2..254
        256 + ev,                               # mt1: real k = 256..510
        np.concatenate([[512], 513 + ev[1:]]),  # mt2: [real 512, imag 2..254]
        513 + 256 + ev,                         # mt3: imag k = 256..510
        od,                                     # mt4: real k = 1,3..255
        256 + od,                               # mt5: real k = 257..511
        513 + od,                               # mt6: imag k = 1..255
        513 + 256 + od,                         # mt7: imag k = 257..511
    ]
    w2 = np.empty((512, MT * 128), dtype=np.float32)  # [k, m]
    for mt, chs in enumerate(ch_tiles):
        assert len(chs) == 128, (mt, len(chs))
        w2[:, mt * 128 : (mt + 1) * 128] = fb[chs, :].T
    w_send = np.ascontiguousarray(
        w2.reshape(KC, 128, MT, 128).transpose(1, 0, 2, 3).reshape(128, -1)
    )
    return w_send


def pack_winv():
    win = _win512().astype(np.float64)
    winv = np.empty((128, 12), dtype=np.float32)
    for kc in range(4):
        seg = win[kc * 128 : (kc + 1) * 128]
        winv[:, kc] = seg
        winv[:, 4 + kc] = 1.0 - seg
        winv[:, 8 + kc] = -(1.0 - seg)
    return winv


def shard_audio(audio):
    """Full audio [15360000] -> per-core transposed blocks [512, F+1]."""
    padded = np.pad(np.asarray(audio, dtype=np.float32), PAD, mode="reflect")
    need = HOP * ((N_CORES - 1) * F + F + 1)  # samples covering all core spans
    ext = np.zeros(need, dtype=np.float32)
    ext[: padded.shape[0]] = padded
    shards = []
    for c in range(N_CORES):
        lo = HOP * c * F
        blk = ext[lo : lo + HOP * (F + 1)].reshape(F + 1, HOP)
        shards.append(np.ascontiguousarray(blk.T))  # [512, F+1]
    return shards


def kernel(audio, forward_basis):
    nc = build_stft_nc()
    w_send = pack_weights(forward_basis)
    winv = pack_winv()
    shards = shard_audio(audio)
    in_maps = [
        {"audio_t": shards[c], "w": w_send, "winv": winv} for c in range(N_CORES)
    ]
    res = run_bass_kernel_spmd(nc, in_maps, core_ids=list(range(N_CORES)))
    outs = [r["out"] for r in res.results]  # each [513, F]
    full = np.concatenate(outs, axis=1)[:, :T_FRAMES]
    return full[None, :, :].astype(np.float32)


# revision 17
# speedup vs baseline: 2.2681x; 2.2681x over previous
"""DecomposedSTFT power-spectrum kernel for 8 Trainium2 NeuronCores.

Reference computation:
    padded = reflect_pad(audio, 512)                       # [15361024]
    frame_t = padded[512*t : 512*t + 1024], t = 0..30000   # hop 512, win 1024
    ft[ch, t] = sum_n basis[ch, n] * frame_t[n]            # basis [1026, 1024]
    out[k, t] = ft[k, t]^2 + ft[513+k, t]^2                # k = 0..512

Structure exploited:
  * Window fold: basis[ch] = fb[ch] * win with periodic Hann win, so
    win[n+512] = 1 - win[n] and fb[ch, n+512] = (-1)^k fb[ch, n].  Hence
        ft[ch] = sum_{n<512} fb[ch, n] * (u[n] if k even else d[n])
        u_t[n] = win[n] x_t[n] + (1-win[n]) x_t[n+512]
        d_t[n] = win[n] x_t[n] - (1-win[n]) x_t[n+512]
    halving the matmul contraction from 1024 to 512.  fb is recovered from
    the input basis exactly: fb[ch,n] = basis[ch,n] +- basis[ch,n+512].
    u/d are built on-device (ACT scale-copy + DVE fused multiply-adds) so
    the audio crosses PCIe/HBM exactly once.
  * imag rows 0 and 512 are identically zero, so the 1026 channels are
    exactly 1024 nonzero rows = 8 M-tiles of 128, grouped by bin parity:
    mt 0-3 read u (even bins), mt 4-7 read d (odd bins).  Power =
    square + square partitionwise with two single-row fixups (k=0, k=512).
  * hop = win/2: audio laid out as 512-sample blocks (transposed host-side
    to [512, n_blocks]) makes x[n+512] a 1-column shift of x[n].
  * float32r matmuls: 1 column/cycle for even N >= 256, full fp32 numerics.
  * DMAs carry a ~2us serialized fixed cost each, so they are batched:
    audio arrives in 4 whole-iteration transfers, outputs leave in
    2-chunk-wide strips, split across both HWDGE rings (sync + scalar).

Sharding: frames split across 8 cores (3760 frames/core, surplus frames
cropped on gather); forward_basis replicated.
"""

import contextlib

import numpy as np

import concourse.bass as bass
import concourse.mybir as mybir
import concourse.tile as tile
from concourse import bacc
from concourse.bass_utils import run_bass_kernel_spmd

FL = 1024
HOP = 512
CUT = 513  # output rows
PAD = 512
L = 15360000
T_FRAMES = 30001

N_CORES = 8
F = 3760            # frames per core (8*3760 = 30080 >= 30001)
CHUNK = 470         # frames per N-chunk; must be EVEN (f32r ISA restriction),
                    # >= 256 (f32r full rate), <= 512 (one PSUM bank)
N_CHUNKS = 8
GRP = 4             # chunks per output strip (out-DMA batching)
KC = 4              # K chunks of 128 (folded contraction dim 512)
MT = 8              # M tiles of 128 (1024 nonzero basis rows)

F32R = mybir.dt.float32r
F32 = mybir.dt.float32

# (mt_a, mt_b, out_row_offset): power pairs and the DRAM rows they produce
# (row = offset + 2*p).  mt 0/1 = real even bins, 2/3 = [Nyq, imag even],
# 4/5 = real odd, 6/7 = imag odd.
PAIRS = [(0, 2, 0), (1, 3, 256), (4, 6, 1), (5, 7, 257)]


def build_stft_nc(F=F, chunk=CHUNK, n_chunks=N_CHUNKS, repeat=1, out_mode="swdge_strip"):
    """Bass program: audio_t [512, F+1] x w [128, 4096] -> out [513, F].

    repeat > 1 wraps the compute in a hardware loop redoing identical work;
    used only for wall-clock benchmarking (Dwall between two repeat counts
    isolates per-iteration HW time from dispatch/transfer overhead).
    """
    assert chunk * n_chunks == F
    nc = bacc.Bacc("TRN2", target_bir_lowering=False, debug=False)
    audio_t = nc.dram_tensor("audio_t", [512, F + 1], F32R, kind="ExternalInput")
    w_dram = nc.dram_tensor("w", [128, KC * MT * 128], F32R, kind="ExternalInput")
    winv_dram = nc.dram_tensor("winv", [128, 12], F32, kind="ExternalInput")
    out_dram = nc.dram_tensor("out", [CUT, F], F32, kind="ExternalOutput")

    def out_ap(row_off, row_step, nrows, col_off, ncols):
        return bass.AP(
            tensor=out_dram,
            offset=row_off * F + col_off,
            ap=[[row_step * F, nrows], [1, ncols]],
        )

    with tile.TileContext(nc) as tc:
        with (
            tc.tile_pool(name="wpool", bufs=1) as wpool,
            tc.tile_pool(name="apool", bufs=3) as apool,
            tc.tile_pool(name="zpool", bufs=3) as zpool,
            tc.tile_pool(name="udpool", bufs=3) as udpool,
            tc.tile_pool(name="spool", bufs=8) as spool,
            tc.tile_pool(name="opool", bufs=2) as opool,
            tc.tile_pool(name="npool", bufs=1) as npool,
            tc.tile_pool(name="psum", bufs=8, space="PSUM") as pp,
        ):
            winv = wpool.tile([128, 12], F32)
            nc.sync.dma_start(out=winv, in_=winv_dram[:, :])
            wt = wpool.tile([128, KC * MT * 128], F32R)
            nc.sync.dma_start(out=wt, in_=w_dram[:, :])

            loop_ctx = (
                tc.For_i(0, repeat, 1, hint_engines=(mybir.EngineType.PE,))
                if repeat > 1
                else contextlib.nullcontext()
            )
            with loop_ctx:
                nyq_full = npool.tile([1, F], F32, tag="nyq")
                ostrip = {}
                for n in range(n_chunks):
                    g = n % GRP  # position within the output strip
                    if g == 0:
                        for pi in range(4):
                            ostrip[pi] = opool.tile(
                                [128, GRP * chunk], F32,
                                tag=f"o{pi}", name=f"ostrip{pi}",
                            )

                    c0 = n * chunk
                    ud = {}
                    for kc in range(KC):
                        x = apool.tile([128, chunk + 1], F32R, tag=f"x{kc}",
                                       name=f"x{kc}")
                        nc.sync.dma_start(
                            out=x,
                            in_=audio_t[
                                kc * 128 : (kc + 1) * 128, c0 : c0 + chunk + 1
                            ],
                        )
                        x0 = x[:, 0:chunk]
                        x1 = x[:, 1 : chunk + 1]
                        # z = win * x[n]; u = z + (1-win) x[n+512];
                        # d = z - (1-win) x[n+512]
                        z = zpool.tile([128, chunk], F32R, tag="z")
                        nc.scalar.activation(
                            z,
                            x0,
                            mybir.ActivationFunctionType.Copy,
                            scale=winv[:, kc : kc + 1],
                        )
                        u = udpool.tile([128, chunk], F32R, tag=f"u{kc}")
                        d = udpool.tile([128, chunk], F32R, tag=f"d{kc}")
                        nc.vector.scalar_tensor_tensor(
                            u, x1, winv[:, 4 + kc : 5 + kc], z,
                            mybir.AluOpType.mult, mybir.AluOpType.add,
                        )
                        nc.vector.scalar_tensor_tensor(
                            d, x1, winv[:, 8 + kc : 9 + kc], z,
                            mybir.AluOpType.mult, mybir.AluOpType.add,
                        )
                        ud[kc] = (u, d)

                    sq = [None] * MT
                    for pi, (ma, mb, row_off) in enumerate(PAIRS):
                        for mt in (ma, mb):
                            p = pp.tile([128, chunk], F32)
                            for kc in range(KC):
                                rhs = ud[kc][0] if mt < 4 else ud[kc][1]
                                nc.tensor.matmul(
                                    p,
                                    wt[
                                        :,
                                        (kc * MT + mt) * 128 : (kc * MT + mt + 1) * 128,
                                    ],
                                    rhs,
                                    start=(kc == 0),
                                    stop=(kc == KC - 1),
                                )
                            s = spool.tile([128, chunk], F32, tag="sq")
                            nc.scalar.activation(
                                s, p, mybir.ActivationFunctionType.Square
                            )
                            sq[mt] = s

                        o = ostrip[pi][:, g * chunk : (g + 1) * chunk]
                        nc.vector.tensor_add(o, sq[ma], sq[mb])
                        if row_off == 0:
                            # row 0: imag_0 == 0; slot (2, p=0) actually holds
                            # the Nyquist real row -> out[0] = real_0^2 only
                            nc.vector.tensor_copy(o[0:1, :], sq[0][0:1, :])
                            # row 512: Nyquist power = (fb_512 . u)^2
                            nc.vector.tensor_copy(
                                nyq_full[:, c0 : c0 + chunk], sq[2][0:1, :]
                            )
                        if out_mode == "sp_chunk":
                            nc.sync.dma_start(
                                out=out_ap(row_off, 2, 128, c0, chunk),
                                in_=o,
                            )
                        elif g == GRP - 1 or n == n_chunks - 1:
                            # strip complete: one DMA per pair on the SWDGE
                            # (gpsimd) queue so blocked output waits never
                            # stall input loads or compute engines
                            nc.gpsimd.dma_start(
                                out=out_ap(
                                    row_off, 2, 128, (n - g) * chunk,
                                    (g + 1) * chunk,
                                ),
                                in_=ostrip[pi][:, 0 : (g + 1) * chunk],
                            )
                nc.gpsimd.dma_start(out=out_ap(512, 1, 1, 0, F), in_=nyq_full)
    nc.compile()
    return nc


def _win512():
    n = np.arange(512)
    return (0.5 * (1.0 - np.cos(2.0 * np.pi * n / FL))).astype(np.float32)


def pack_weights(forward_basis):
    """[1026, 1, 1024] conv basis -> [128, 4096] folded lhsT tiles.

    fb[ch, n] = basis[ch, n] + (-1)^k basis[ch, n+512]  (exact unfold of the
    periodic-Hann window).  M layout groups bins by parity; imag rows 0/512
    (identically zero) are dropped and the Nyquist real row takes slot
    (mt=2, p=0).
    """
    basis = np.asarray(forward_basis, dtype=np.float32)[:, 0, :]  # [1026, 1024]
    k_of_ch = np.concatenate([np.arange(513), np.arange(513)])  # channel -> bin
    sign = np.where(k_of_ch % 2 == 0, 1.0, -1.0).astype(np.float32)
    fb = basis[:, :512] + sign[:, None] * basis[:, 512:]  # [1026, 512]

    ev = np.arange(0, 256, 2)
    od = np.arange(1, 256, 2)
    ch_tiles = [
        ev,                                     # mt0: real k = 0,2..254
        256 + ev,                               # mt1: real k = 256..510
        np.concatenate([[512], 513 + ev[1:]]),  # mt2: [real 512, imag 2..254]
        513 + 256 + ev,                         # mt3: imag k = 256..510
        od,                                     # mt4: real k = 1,3..255
        256 + od,                               # mt5: real k = 257..511
        513 + od,                               # mt6: imag k = 1..255
        513 + 256 + od,                         # mt7: imag k = 257..511
    ]
    w2 = np.empty((512, MT * 128), dtype=np.float32)  # [k, m]
    for mt, chs in enumerate(ch_tiles):
        assert len(chs) == 128, (mt, len(chs))
        w2[:, mt * 128 : (mt + 1) * 128] = fb[chs, :].T
    w_send = np.ascontiguousarray(
        w2.reshape(KC, 128, MT, 128).transpose(1, 0, 2, 3).reshape(128, -1)
    )
    return w_send


def pack_winv():
    win = _win512().astype(np.float64)
    winv = np.empty((128, 12), dtype=np.float32)
    for kc in range(4):
        seg = win[kc * 128 : (kc + 1) * 128]
        winv[:, kc] = seg
        winv[:, 4 + kc] = 1.0 - seg
        winv[:, 8 + kc] = -(1.0 - seg)
    return winv


def shard_audio(audio):
    """Full audio [15360000] -> per-core transposed blocks [512, F+1]."""
    padded = np.pad(np.asarray(audio, dtype=np.float32), PAD, mode="reflect")
    need = HOP * ((N_CORES - 1) * F + F + 1)  # samples covering all core spans
    ext = np.zeros(need, dtype=np.float32)
    ext[: padded.shape[0]] = padded
    shards = []
    for c in range(N_CORES):
        lo = HOP * c * F
        blk = ext[lo : lo + HOP * (F + 1)].reshape(F + 1, HOP)
        shards.append(np.ascontiguousarray(blk.T))  # [512, F+1]
    return shards


def kernel(audio, forward_basis):
    nc = build_stft_nc()
    w_send = pack_weights(forward_basis)
    winv = pack_winv()
    shards = shard_audio(audio)
    in_maps = [
        {"audio_t": shards[c], "w": w_send, "winv": winv} for c in range(N_CORES)
    ]
    res = run_bass_kernel_spmd(nc, in_maps, core_ids=list(range(N_CORES)))
    outs = [r["out"] for r in res.results]  # each [513, F]
    full = np.concatenate(outs, axis=1)[:, :T_FRAMES]
    return full[None, :, :].astype(np.float32)
